# revision 41
# baseline (speedup 1.0000x reference)
"""Trainium2 Bass kernel for nn_CrossAttention_15006615733765 (raw Bass, no Tile).

Mathematical structure: the reference broadcasts a per-batch context vector
(B, CTX_DIM) to every spatial position before projecting to K/V.  All keys
within a batch are therefore identical, softmax over the key axis is exactly
uniform, and the attention output equals V itself.  The module collapses to

    out[b, c, h, w] = ((context[b] @ Wv) @ Wo + bo)[c]

independent of x, Wq and Wk (exact in infinite precision).  The kernel
computes the two small matmuls on the tensor engine and materializes the
broadcast output shard per core, sharding the 512 output channels across 8
cores.

v2: bf16 on every DMA-heavy path.  Wv/ctx/Wo/sel stream in as bf16 (half
the f32 bytes, single-pass PE matmuls instead of fp32 LOW_HIGH passes), and
the 2304x-broadcast output shard is stored as bf16 (host upcasts to f32).
Measured rel err vs the f32 reference is ~2e-3 (tolerance 2e-2).

Engine plan (raw Bass, hand-placed semaphores):
  Sync   : one DMA for (Wv chunks 0-1 | ctx) packed per-partition, Wv pair
           4-5, then the single broadcast output store (one queue is as
           fast as two -- descriptor generation is a shared resource)
  Scalar : Wv pair 2-3 + id + wo + sel/bias pack
  Tensor : HAM warmup -> stage1 (6 matmuls, t = ctx @ Wv) -> 4 transposes
           -> stage2 (4 matmuls, y = tT @ Wo) -> 4 selector matmuls that
           broadcast y+bias across all 128 partitions (into two PSUM banks
           so the rep casts pipeline against the b=2,3 matmuls)
  Vector : PSUM -> SBUF copies between PE stages; replicated-store source
           built by two tensor_copys reading the selector PSUM banks
           through a stride-0 dup dimension (6 replicas -> 3 KiB bf16
           store descriptors, p-major so each partition's 18 output rows
           are contiguous in DRAM; row order is free since every row of
           the output shard is identical)
  GpSimd : unused (block exits with no_gpsimd_drain; Pool cannot read
           PSUM -- the BIR verifier rejects it)
HW lessons encoded here (CoreSim models all of these as fine):
  - scalar_tensor_tensor / Activation-engine copies reading PSUM through
    broadcast APs return garbage or abort on HW; only DVE tensor_copy
    handles PSUM + broadcast correctly.
  - A DVE read of a PSUM bank while the PE writes other columns of the
    SAME bank corrupts data -> transposes land in two separate banks.
  - Each dma_start costs ~0.7us engine issue + ~0.8us to first byte, so
    fewer/larger DMAs win; 2-3 KiB descriptors sustain ~200 GB/s.
The store has no explicit completion wait: the block-exit DRAIN on the
issuing engine waits for its HWDGE queue, so the walrus semaphore-reset
epilogue overlaps the output transfer.
"""

import numpy as np
import ml_dtypes

import concourse.bacc as bacc
import concourse.mybir as mybir
from concourse.bass_utils import run_bass_kernel_spmd

B, DIM, CTX_DIM = 4, 512, 768
H = W = 48
NPOS = H * W
NCORES = 8
CPC = DIM // NCORES
P = 128
KC = CTX_DIM // P
KD = DIM // P
ROW = B * CPC
NDUP = 6  # replicas per partition -> 3 KiB bf16 DMA descriptors
SELW = B * P + CPC  # sel rows + o5/bias columns
F32 = mybir.dt.float32
BF16 = mybir.dt.bfloat16
BF16NP = ml_dtypes.bfloat16

_CACHE: dict = {}
NWARM = 5  # HAM warmup matmuls; set to 0 before _get_nc() for CoreSim runs


def _build_nc():
    nc = bacc.Bacc("TRN2", target_bir_lowering=False, debug=False, num_devices=NCORES)

    wac = nc.dram_tensor("wac", [P, 2 * DIM + KC * B], BF16, kind="ExternalInput")
    wvc = nc.dram_tensor("wvc", [P, KC, DIM], BF16, kind="ExternalInput")
    woc = nc.dram_tensor("woc", [P, KD, CPC], BF16, kind="ExternalInput")
    idc = nc.dram_tensor("idc", [B, B], F32, kind="ExternalInput")
    smlc = nc.dram_tensor("smlc", [B + 1, SELW], BF16, kind="ExternalInput")
    outd = nc.dram_tensor("outd", [NPOS, ROW], BF16, kind="ExternalOutput")

    wa_sb = nc.alloc_sbuf_tensor("wa_sb", [P, 2 * DIM + KC * B], BF16).ap()
    wv01_v = wa_sb[:, : 2 * DIM].rearrange("p (k d) -> p k d", k=2)
    ctx_v = wa_sb[:, 2 * DIM :].rearrange("p (k b) -> p k b", k=KC)
    wv_sb = nc.alloc_sbuf_tensor("wv_sb", [P, KC, DIM], BF16).ap()
    wo_sb = nc.alloc_sbuf_tensor("wo_sb", [P, KD, CPC], BF16).ap()
    id_sb = nc.alloc_sbuf_tensor("id_sb", [B, B], F32).ap()
    # sml: [:, :B*P] selector rows (viewed [B+1, B, P]); [:, B*P:] o5
    # (rows 0..3 y from PSUM, row 4 the preloaded bias)
    sml_sb = nc.alloc_sbuf_tensor("sml_sb", [B + 1, SELW], BF16).ap()
    sel_v = sml_sb[:, : B * P].rearrange("q (b p) -> q b p", b=B)
    o5_v = sml_sb[:, B * P :]
    t_sb = nc.alloc_sbuf_tensor("t_sb", [B, DIM], F32).ap()
    tT_sb = nc.alloc_sbuf_tensor("tT_sb", [P, KD, B], BF16).ap()
    rep_sb = nc.alloc_sbuf_tensor("rep_sb", [P, NDUP, ROW], BF16).ap()

    pt = nc.alloc_psum_tensor("pt", [B, DIM], F32).ap()
    ptT0 = nc.alloc_psum_tensor("ptT0", [P, KD // 2, B], F32).ap()
    ptT1 = nc.alloc_psum_tensor("ptT1", [P, KD // 2, B], F32).ap()
    po = nc.alloc_psum_tensor("po", [B, CPC], F32).ap()
    prep0 = nc.alloc_psum_tensor("prep0", [P, 2, CPC], F32).ap()
    prep1 = nc.alloc_psum_tensor("prep1", [P, 2, CPC], F32).ap()
    pwarm = nc.alloc_psum_tensor("pwarm", [B, DIM], F32).ap()

    from contextlib import ExitStack

    with ExitStack() as stack:
        s_wa = stack.enter_context(nc.semaphore("s_wa"))
        s_wv23 = stack.enter_context(nc.semaphore("s_wv23"))
        s_wv45 = stack.enter_context(nc.semaphore("s_wv45"))
        s_id = stack.enter_context(nc.semaphore("s_id"))
        s_wo = stack.enter_context(nc.semaphore("s_wo"))
        s_sml = stack.enter_context(nc.semaphore("s_sml"))
        s_s1 = stack.enter_context(nc.semaphore("s_s1"))
        s_tcp = stack.enter_context(nc.semaphore("s_tcp"))
        s_tT = stack.enter_context(nc.semaphore("s_tT"))
        s_st2 = stack.enter_context(nc.semaphore("s_st2"))
        s_o5 = stack.enter_context(nc.semaphore("s_o5"))
        s_sel01 = stack.enter_context(nc.semaphore("s_sel01"))
        s_sel23 = stack.enter_context(nc.semaphore("s_sel23"))
        s_rep = stack.enter_context(nc.semaphore("s_rep"))
        s_out = stack.enter_context(nc.semaphore("s_out"))

        # store view: pos = r*(P*NDUP) + p*NDUP + d, so each partition
        # contributes NDUP consecutive 512-byte rows = 3 KiB contiguous.
        out_view = outd.rearrange("(p r d) n -> p r (d n)", p=P, d=NDUP)
        src_view = (
            rep_sb.rearrange("p d n -> p (d n)")[:, None, :]
            .broadcast_to((P, NPOS // (NDUP * P), NDUP * ROW))
        )
        HP = P // 2

        with nc.Block(no_gpsimd_drain=True) as block:

            @block.sync
            def _(sync):
                sync.dma_start(out=wa_sb[:], in_=wac[:]).then_inc(s_wa, 16)
                sync.dma_start(
                    out=wv_sb[:, 4:6, :], in_=wvc[:, 4:6, :]
                ).then_inc(s_wv45, 16)
                sync.wait_ge(s_rep, 2)
                sync.dma_start(out=out_view[:], in_=src_view[:]).then_inc(
                    s_out, 16
                )

            @block.scalar
            def _(scalar):
                scalar.dma_start(
                    out=wv_sb[:, 2:4, :], in_=wvc[:, 2:4, :]
                ).then_inc(s_wv23, 16)
                scalar.dma_start(out=id_sb[:], in_=idc[:]).then_inc(s_id, 16)
                scalar.dma_start(out=wo_sb[:], in_=woc[:]).then_inc(s_wo, 16)
                scalar.dma_start(out=sml_sb[:], in_=smlc[:]).then_inc(s_sml, 16)


            @block.tensor
            def _(tensor):
                # HAM warmup: ungated dummy matmuls (garbage SBUF data,
                # scratch PSUM) keep the PE busy from boot until the first
                # Wv chunk lands, ramping the PE clock from 1.2 to 2.4 GHz.
                for _w in range(NWARM):
                    nc.tensor.matmul(
                        pwarm[:],
                        ctx_v[:, 0, :],
                        wv_sb[:, KC - 1, :],
                        start=(_w == 0),
                        stop=(_w == NWARM - 1),
                    )
                tensor.wait_ge(s_wa, 16)
                gates = [None, None, (s_wv23, 16), None, (s_wv45, 16), None]
                for k in range(KC):
                    if gates[k] is not None:
                        tensor.wait_ge(*gates[k])
                    rhs = wv01_v[:, k, :] if k < 2 else wv_sb[:, k, :]
                    ins = nc.tensor.matmul(
                        pt[:],
                        ctx_v[:, k, :],
                        rhs,
                        start=(k == 0),
                        stop=(k == KC - 1),
                    )
                ins.then_inc(s_s1, 1)

                tensor.wait_ge(s_id, 16)
                tensor.wait_ge(s_tcp, 1)
                for m in range(KD // 2):
                    nc.tensor.transpose(
                        ptT0[:, m, :], t_sb[:, m * P : (m + 1) * P], id_sb[:]
                    ).then_inc(s_st2, 1)
                tensor.wait_ge(s_tcp, 2)
                for m in range(KD // 2, KD):
                    nc.tensor.transpose(
                        ptT1[:, m - KD // 2, :], t_sb[:, m * P : (m + 1) * P],
                        id_sb[:],
                    ).then_inc(s_st2, 1)

                # stage2: y = tT.T @ Wo, accumulated over the 4 d-chunks
                tensor.wait_ge(s_wo, 16)
                tensor.wait_ge(s_tT, 1)
                for m in range(KD // 2):
                    nc.tensor.matmul(
                        po[:],
                        tT_sb[:, m, :],
                        wo_sb[:, m, :],
                        start=(m == 0),
                        stop=False,
                    )
                tensor.wait_ge(s_tT, 2)
                for m in range(KD // 2, KD):
                    ins = nc.tensor.matmul(
                        po[:],
                        tT_sb[:, m, :],
                        wo_sb[:, m, :],
                        start=False,
                        stop=(m == KD - 1),
                    )
                ins.then_inc(s_st2, 1)

                # selector: prep[p, b, :] = y[b, :] + bias, all partitions
                tensor.wait_ge(s_o5, 1)
                for b in range(B):
                    pdst = prep0 if b < 2 else prep1
                    ins = nc.tensor.matmul(
                        pdst[:, b % 2, :],
                        sel_v[:, b, :],
                        o5_v[:, :],
                        start=True,
                        stop=True,
                    )
                    if b == 1:
                        ins.then_inc(s_sel01, 1)
                ins.then_inc(s_sel23, 1)

            @block.vector
            def _(vector):
                vector.wait_ge(s_s1, 1)
                nc.vector.tensor_copy(
                    t_sb[:, : DIM // 2], pt[:, : DIM // 2]
                ).then_inc(s_tcp, 1)
                nc.vector.tensor_copy(
                    t_sb[:, DIM // 2 :], pt[:, DIM // 2 :]
                ).then_inc(s_tcp, 1)
                vector.wait_ge(s_st2, 2)
                nc.vector.tensor_copy(
                    tT_sb[:, : KD // 2, :], ptT0[:]
                ).then_inc(s_tT, 1)
                vector.wait_ge(s_st2, 4)
                nc.vector.tensor_copy(
                    tT_sb[:, KD // 2 :, :], ptT1[:]
                ).then_inc(s_tT, 1)
                # y rows of o5 (bias row 4 preloaded by the smlc DMA; wait
                # on it so this write is ordered after the DMA's)
                vector.wait_ge(s_st2, 5)
                vector.wait_ge(s_sml, 16)
                nc.vector.tensor_copy(o5_v[:B, :], po[:]).then_inc(s_o5, 1)
                # store source: 6 bias-included replicas per partition, one
                # copy reading prep PSUM through a stride-0 dup dimension
                vector.wait_ge(s_sel01, 1)
                nc.vector.tensor_copy(
                    rep_sb[:, :, : ROW // 2],
                    prep0.rearrange("p b c -> p (b c)")[:, None, :]
                    .broadcast_to((P, NDUP, ROW // 2)),
                ).then_inc(s_rep, 1)
                vector.wait_ge(s_sel23, 1)
                nc.vector.tensor_copy(
                    rep_sb[:, :, ROW // 2 :],
                    prep1.rearrange("p b c -> p (b c)")[:, None, :]
                    .broadcast_to((P, NDUP, ROW // 2)),
                ).then_inc(s_rep, 1)

    nc.compile()
    return nc


def _get_nc():
    if "nc" not in _CACHE:
        _CACHE["nc"] = _build_nc()
    return _CACHE["nc"]


def _prepare_in_maps(context, Wv, Wo, bo):
    context = np.ascontiguousarray(context, dtype=np.float32)
    Wv = np.ascontiguousarray(Wv, dtype=np.float32)
    Wo = np.ascontiguousarray(Wo, dtype=np.float32)
    bo = np.ascontiguousarray(bo, dtype=np.float32)

    ctx_pack = context.T.reshape(KC, P, B).transpose(1, 0, 2).reshape(P, KC * B)
    wv_all = Wv.reshape(KC, P, DIM).transpose(1, 0, 2)
    wac = np.concatenate(
        [wv_all[:, :2, :].reshape(P, 2 * DIM), ctx_pack], axis=1
    ).astype(BF16NP)
    wvc = np.ascontiguousarray(wv_all).astype(BF16NP)
    wo_chunk = Wo.reshape(KD, P, DIM).transpose(1, 0, 2)
    idc = np.eye(B, dtype=np.float32)

    sel = np.zeros((B + 1, B, P), dtype=np.float32)
    for b in range(B):
        sel[b, b, :] = 1.0
        sel[B, b, :] = 1.0

    in_maps = []
    for i in range(NCORES):
        woc = np.ascontiguousarray(
            wo_chunk[:, :, i * CPC : (i + 1) * CPC]
        ).astype(BF16NP)
        smlc = np.zeros((B + 1, SELW), dtype=np.float32)
        smlc[:, : B * P] = sel.reshape(B + 1, B * P)
        smlc[B, B * P :] = bo[i * CPC : (i + 1) * CPC]
        in_maps.append(
            {
                "wac": wac,
                "wvc": wvc,
                "woc": woc,
                "idc": idc,
                "smlc": smlc.astype(BF16NP),
            }
        )
    return in_maps


def _unshard(results):
    shards = np.stack(
        [np.asarray(r["outd"]).astype(np.float32) for r in results], axis=0
    )
    shards = shards.reshape(NCORES, NPOS, B, CPC)
    out = shards.transpose(2, 0, 3, 1).reshape(B, DIM, H, W)
    return np.ascontiguousarray(out)


def kernel(x, context, Wq, Wk, Wv, Wo, bo):
    del x, Wq, Wk
    nc = _get_nc()
    in_maps = _prepare_in_maps(context, Wv, Wo, bo)
    results = run_bass_kernel_spmd(nc, in_maps, list(range(NCORES))).results
    return _unshard(results)


# revision 42
# speedup vs baseline: 1.0250x; 1.0250x over previous
"""Trainium2 Bass kernel for nn_CrossAttention_15006615733765 (raw Bass, no Tile).

Mathematical structure: the reference broadcasts a per-batch context vector
(B, CTX_DIM) to every spatial position before projecting to K/V.  All keys
within a batch are therefore identical, softmax over the key axis is exactly
uniform, and the attention output equals V itself.  The module collapses to

    out[b, c, h, w] = ((context[b] @ Wv) @ Wo + bo)[c]

independent of x, Wq and Wk (exact in infinite precision).  The kernel
computes the two small matmuls on the tensor engine and materializes the
broadcast output shard per core, sharding the 512 output channels across 8
cores.

v2: bf16 on every DMA-heavy path.  Wv/ctx/Wo/sel stream in as bf16 (half
the f32 bytes, single-pass PE matmuls instead of fp32 LOW_HIGH passes), and
the 2304x-broadcast output shard is stored as bf16 (host upcasts to f32).
Measured rel err vs the f32 reference is ~2e-3 (tolerance 2e-2).

Engine plan (raw Bass, hand-placed semaphores):
  Sync   : one DMA for (Wv chunks 0-1 | ctx) packed per-partition, Wv pair
           4-5, then the single broadcast output store (one queue is as
           fast as two -- descriptor generation is a shared resource)
  Scalar : Wv pair 2-3 + id + wo + sel/bias pack
  Tensor : HAM warmup -> stage1 (6 matmuls, t = ctx @ Wv) -> 4 transposes
           -> stage2 (4 matmuls, y = tT @ Wo) -> 4 selector matmuls that
           broadcast y+bias across all 128 partitions (into two PSUM banks
           so the rep casts pipeline against the b=2,3 matmuls)
  Vector : PSUM -> SBUF copies between PE stages; replicated-store source
           built by two tensor_copys reading the selector PSUM banks
           through a stride-0 dup dimension (6 replicas -> 3 KiB bf16
           store descriptors, p-major so each partition's 18 output rows
           are contiguous in DRAM; row order is free since every row of
           the output shard is identical)
  GpSimd : unused (block exits with no_gpsimd_drain; Pool cannot read
           PSUM -- the BIR verifier rejects it)
HW lessons encoded here (CoreSim models all of these as fine):
  - scalar_tensor_tensor / Activation-engine copies reading PSUM through
    broadcast APs return garbage or abort on HW; only DVE tensor_copy
    handles PSUM + broadcast correctly.
  - A DVE read of a PSUM bank while the PE writes other columns of the
    SAME bank corrupts data -> transposes land in two separate banks.
  - Each dma_start costs ~0.7us engine issue + ~0.8us to first byte, so
    fewer/larger DMAs win; 2-3 KiB descriptors sustain ~200 GB/s.
The store has no explicit completion wait: the block-exit DRAIN on the
issuing engine waits for its HWDGE queue, so the walrus semaphore-reset
epilogue overlaps the output transfer.
"""

import numpy as np
import ml_dtypes

import concourse.bacc as bacc
import concourse.mybir as mybir
from concourse.bass_utils import run_bass_kernel_spmd

B, DIM, CTX_DIM = 4, 512, 768
H = W = 48
NPOS = H * W
NCORES = 8
CPC = DIM // NCORES
P = 128
KC = CTX_DIM // P
KD = DIM // P
ROW = B * CPC
NDUP = 6  # replicas per partition -> 3 KiB bf16 DMA descriptors
SELW = B * P + CPC  # sel rows + o5/bias columns
F32 = mybir.dt.float32
BF16 = mybir.dt.bfloat16
BF16NP = ml_dtypes.bfloat16

_CACHE: dict = {}
NWARM = 5  # HAM warmup matmuls; set to 0 before _get_nc() for CoreSim runs


def _build_nc():
    nc = bacc.Bacc("TRN2", target_bir_lowering=False, debug=False, num_devices=NCORES)

    wac = nc.dram_tensor("wac", [P, 2 * DIM + KC * B], BF16, kind="ExternalInput")
    wvc = nc.dram_tensor("wvc", [P, KC, DIM], BF16, kind="ExternalInput")
    woc = nc.dram_tensor("woc", [P, KD, CPC], BF16, kind="ExternalInput")
    idc = nc.dram_tensor("idc", [B, B], F32, kind="ExternalInput")
    smlc = nc.dram_tensor("smlc", [B + 1, SELW], BF16, kind="ExternalInput")
    outd = nc.dram_tensor("outd", [NPOS, ROW], BF16, kind="ExternalOutput")

    wa_sb = nc.alloc_sbuf_tensor("wa_sb", [P, 2 * DIM + KC * B], BF16).ap()
    wv01_v = wa_sb[:, : 2 * DIM].rearrange("p (k d) -> p k d", k=2)
    ctx_v = wa_sb[:, 2 * DIM :].rearrange("p (k b) -> p k b", k=KC)
    wv_sb = nc.alloc_sbuf_tensor("wv_sb", [P, KC, DIM], BF16).ap()
    wo_sb = nc.alloc_sbuf_tensor("wo_sb", [P, KD, CPC], BF16).ap()
    id_sb = nc.alloc_sbuf_tensor("id_sb", [B, B], F32).ap()
    # sml: [:, :B*P] selector rows (viewed [B+1, B, P]); [:, B*P:] o5
    # (rows 0..3 y from PSUM, row 4 the preloaded bias)
    sml_sb = nc.alloc_sbuf_tensor("sml_sb", [B + 1, SELW], BF16).ap()
    sel_v = sml_sb[:, : B * P].rearrange("q (b p) -> q b p", b=B)
    o5_v = sml_sb[:, B * P :]
    t_sb = nc.alloc_sbuf_tensor("t_sb", [B, DIM], F32).ap()
    tT_sb = nc.alloc_sbuf_tensor("tT_sb", [P, KD, B], BF16).ap()
    rep_sb = nc.alloc_sbuf_tensor("rep_sb", [P, NDUP, ROW], BF16).ap()

    pt = nc.alloc_psum_tensor("pt", [B, DIM], F32).ap()
    ptT0 = nc.alloc_psum_tensor("ptT0", [P, KD // 2, B], F32).ap()
    ptT1 = nc.alloc_psum_tensor("ptT1", [P, KD // 2, B], F32).ap()
    po = nc.alloc_psum_tensor("po", [B, CPC], F32).ap()
    prep0 = nc.alloc_psum_tensor("prep0", [P, 2, CPC], F32).ap()
    prep1 = nc.alloc_psum_tensor("prep1", [P, 2, CPC], F32).ap()
    pwarm = nc.alloc_psum_tensor("pwarm", [B, DIM], F32).ap()

    from contextlib import ExitStack

    with ExitStack() as stack:
        # 7 semaphores total: the walrus save/restore pro/epilogue scales
        # with semaphore count, and same-engine increments are ordered, so
        # one monotone counter per producer engine suffices.
        s_wa = stack.enter_context(nc.semaphore("s_wa"))
        s_wv23 = stack.enter_context(nc.semaphore("s_wv23"))
        s_wv45 = stack.enter_context(nc.semaphore("s_wv45"))
        s_const = stack.enter_context(nc.semaphore("s_const"))
        s_p = stack.enter_context(nc.semaphore("s_p"))
        s_v = stack.enter_context(nc.semaphore("s_v"))
        s_out = stack.enter_context(nc.semaphore("s_out"))

        # store view: pos = r*(P*NDUP) + p*NDUP + d, so each partition
        # contributes NDUP consecutive 512-byte rows = 3 KiB contiguous.
        out_view = outd.rearrange("(p r d) n -> p r (d n)", p=P, d=NDUP)
        src_view = (
            rep_sb.rearrange("p d n -> p (d n)")[:, None, :]
            .broadcast_to((P, NPOS // (NDUP * P), NDUP * ROW))
        )
        HP = P // 2

        with nc.Block(no_gpsimd_drain=True) as block:

            @block.sync
            def _(sync):
                sync.dma_start(out=wa_sb[:], in_=wac[:]).then_inc(s_wa, 16)
                sync.dma_start(
                    out=wv_sb[:, 4:6, :], in_=wvc[:, 4:6, :]
                ).then_inc(s_wv45, 16)
                sync.wait_ge(s_v, 7)
                sync.dma_start(out=out_view[:], in_=src_view[:]).then_inc(
                    s_out, 16
                )

            @block.scalar
            def _(scalar):
                scalar.dma_start(
                    out=wv_sb[:, 2:4, :], in_=wvc[:, 2:4, :]
                ).then_inc(s_wv23, 16)
                scalar.dma_start(out=id_sb[:], in_=idc[:]).then_inc(s_const, 16)
                scalar.dma_start(out=wo_sb[:], in_=woc[:]).then_inc(s_const, 16)
                scalar.dma_start(out=sml_sb[:], in_=smlc[:]).then_inc(
                    s_const, 16
                )


            @block.tensor
            def _(tensor):
                # HAM warmup: ungated dummy matmuls (garbage SBUF data,
                # scratch PSUM) keep the PE busy from boot until the first
                # Wv chunk lands, ramping the PE clock from 1.2 to 2.4 GHz.
                for _w in range(NWARM):
                    nc.tensor.matmul(
                        pwarm[:],
                        ctx_v[:, 0, :],
                        wv_sb[:, KC - 1, :],
                        start=(_w == 0),
                        stop=(_w == NWARM - 1),
                    )
                tensor.wait_ge(s_wa, 16)
                gates = [None, None, (s_wv23, 16), None, (s_wv45, 16), None]
                for k in range(KC):
                    if gates[k] is not None:
                        tensor.wait_ge(*gates[k])
                    rhs = wv01_v[:, k, :] if k < 2 else wv_sb[:, k, :]
                    ins = nc.tensor.matmul(
                        pt[:],
                        ctx_v[:, k, :],
                        rhs,
                        start=(k == 0),
                        stop=(k == KC - 1),
                    )
                ins.then_inc(s_p, 1)

                tensor.wait_ge(s_const, 48)
                tensor.wait_ge(s_v, 1)
                for m in range(KD // 2):
                    nc.tensor.transpose(
                        ptT0[:, m, :], t_sb[:, m * P : (m + 1) * P], id_sb[:]
                    ).then_inc(s_p, 1)
                tensor.wait_ge(s_v, 2)
                for m in range(KD // 2, KD):
                    nc.tensor.transpose(
                        ptT1[:, m - KD // 2, :], t_sb[:, m * P : (m + 1) * P],
                        id_sb[:],
                    ).then_inc(s_p, 1)

                # stage2: y = tT.T @ Wo, accumulated over the 4 d-chunks
                tensor.wait_ge(s_v, 3)
                for m in range(KD // 2):
                    nc.tensor.matmul(
                        po[:],
                        tT_sb[:, m, :],
                        wo_sb[:, m, :],
                        start=(m == 0),
                        stop=False,
                    )
                tensor.wait_ge(s_v, 4)
                for m in range(KD // 2, KD):
                    ins = nc.tensor.matmul(
                        po[:],
                        tT_sb[:, m, :],
                        wo_sb[:, m, :],
                        start=False,
                        stop=(m == KD - 1),
                    )
                ins.then_inc(s_p, 1)

                # selector: prep[p, b, :] = y[b, :] + bias, all partitions
                tensor.wait_ge(s_v, 5)
                for b in range(B):
                    pdst = prep0 if b < 2 else prep1
                    ins = nc.tensor.matmul(
                        pdst[:, b % 2, :],
                        sel_v[:, b, :],
                        o5_v[:, :],
                        start=True,
                        stop=True,
                    )
                    if b == 1:
                        ins.then_inc(s_p, 1)
                ins.then_inc(s_p, 1)

            @block.vector
            def _(vector):
                vector.wait_ge(s_p, 1)
                nc.vector.tensor_copy(
                    t_sb[:, : DIM // 2], pt[:, : DIM // 2]
                ).then_inc(s_v, 1)
                nc.vector.tensor_copy(
                    t_sb[:, DIM // 2 :], pt[:, DIM // 2 :]
                ).then_inc(s_v, 1)
                vector.wait_ge(s_p, 3)
                nc.vector.tensor_copy(
                    tT_sb[:, : KD // 2, :], ptT0[:]
                ).then_inc(s_v, 1)
                vector.wait_ge(s_p, 5)
                nc.vector.tensor_copy(
                    tT_sb[:, KD // 2 :, :], ptT1[:]
                ).then_inc(s_v, 1)
                # y rows of o5 (bias row 4 preloaded by the smlc DMA; wait
                # on it so this write is ordered after the DMA's)
                vector.wait_ge(s_p, 6)
                vector.wait_ge(s_const, 48)
                nc.vector.tensor_copy(o5_v[:B, :], po[:]).then_inc(s_v, 1)
                # store source: 6 bias-included replicas per partition, one
                # copy reading prep PSUM through a stride-0 dup dimension
                vector.wait_ge(s_p, 7)
                nc.vector.tensor_copy(
                    rep_sb[:, :, : ROW // 2],
                    prep0.rearrange("p b c -> p (b c)")[:, None, :]
                    .broadcast_to((P, NDUP, ROW // 2)),
                ).then_inc(s_v, 1)
                vector.wait_ge(s_p, 8)
                nc.vector.tensor_copy(
                    rep_sb[:, :, ROW // 2 :],
                    prep1.rearrange("p b c -> p (b c)")[:, None, :]
                    .broadcast_to((P, NDUP, ROW // 2)),
                ).then_inc(s_v, 1)

    nc.compile()
    return nc


def _get_nc():
    if "nc" not in _CACHE:
        _CACHE["nc"] = _build_nc()
    return _CACHE["nc"]


def _prepare_in_maps(context, Wv, Wo, bo):
    context = np.ascontiguousarray(context, dtype=np.float32)
    Wv = np.ascontiguousarray(Wv, dtype=np.float32)
    Wo = np.ascontiguousarray(Wo, dtype=np.float32)
    bo = np.ascontiguousarray(bo, dtype=np.float32)

    ctx_pack = context.T.reshape(KC, P, B).transpose(1, 0, 2).reshape(P, KC * B)
    wv_all = Wv.reshape(KC, P, DIM).transpose(1, 0, 2)
    wac = np.concatenate(
        [wv_all[:, :2, :].reshape(P, 2 * DIM), ctx_pack], axis=1
    ).astype(BF16NP)
    wvc = np.ascontiguousarray(wv_all).astype(BF16NP)
    wo_chunk = Wo.reshape(KD, P, DIM).transpose(1, 0, 2)
    idc = np.eye(B, dtype=np.float32)

    sel = np.zeros((B + 1, B, P), dtype=np.float32)
    for b in range(B):
        sel[b, b, :] = 1.0
        sel[B, b, :] = 1.0

    in_maps = []
    for i in range(NCORES):
        woc = np.ascontiguousarray(
            wo_chunk[:, :, i * CPC : (i + 1) * CPC]
        ).astype(BF16NP)
        smlc = np.zeros((B + 1, SELW), dtype=np.float32)
        smlc[:, : B * P] = sel.reshape(B + 1, B * P)
        smlc[B, B * P :] = bo[i * CPC : (i + 1) * CPC]
        in_maps.append(
            {
                "wac": wac,
                "wvc": wvc,
                "woc": woc,
                "idc": idc,
                "smlc": smlc.astype(BF16NP),
            }
        )
    return in_maps


def _unshard(results):
    shards = np.stack(
        [np.asarray(r["outd"]).astype(np.float32) for r in results], axis=0
    )
    shards = shards.reshape(NCORES, NPOS, B, CPC)
    out = shards.transpose(2, 0, 3, 1).reshape(B, DIM, H, W)
    return np.ascontiguousarray(out)


def kernel(x, context, Wq, Wk, Wv, Wo, bo):
    del x, Wq, Wk
    nc = _get_nc()
    in_maps = _prepare_in_maps(context, Wv, Wo, bo)
    results = run_bass_kernel_spmd(nc, in_maps, list(range(NCORES))).results
    return _unshard(results)


# revision 43
# speedup vs baseline: 1.0572x; 1.0314x over previous
"""Trainium2 Bass kernel for nn_CrossAttention_15006615733765 (raw Bass, no Tile).

Mathematical structure: the reference broadcasts a per-batch context vector
(B, CTX_DIM) to every spatial position before projecting to K/V.  All keys
within a batch are therefore identical, softmax over the key axis is exactly
uniform, and the attention output equals V itself.  The module collapses to

    out[b, c, h, w] = ((context[b] @ Wv) @ Wo + bo)[c]

independent of x, Wq and Wk (exact in infinite precision).  The kernel
computes the two small matmuls on the tensor engine and materializes the
broadcast output shard per core, sharding the 512 output channels across 8
cores.

v2: bf16 on every DMA-heavy path.  Wv/ctx/Wo/sel stream in as bf16 (half
the f32 bytes, single-pass PE matmuls instead of fp32 LOW_HIGH passes), and
the 2304x-broadcast output shard is stored as bf16 (host upcasts to f32).
Measured rel err vs the f32 reference is ~2e-3 (tolerance 2e-2).

Engine plan (raw Bass, hand-placed semaphores):
  Sync   : one DMA for (Wv chunks 0-1 | ctx) packed per-partition, Wv pair
           4-5, then the single broadcast output store (one queue is as
           fast as two -- descriptor generation is a shared resource)
  Scalar : Wv pair 2-3 + id + wo + sel/bias pack
  Tensor : HAM warmup -> stage1 (6 matmuls, t = ctx @ Wv) -> 4 transposes
           -> stage2 (4 matmuls, y = tT @ Wo) -> 4 selector matmuls that
           broadcast y+bias across all 128 partitions (into two PSUM banks
           so the rep casts pipeline against the b=2,3 matmuls)
  Vector : PSUM -> SBUF copies between PE stages; replicated-store source
           built by two tensor_copys reading the selector PSUM banks
           through a stride-0 dup dimension (6 replicas -> 3 KiB bf16
           store descriptors, p-major so each partition's 18 output rows
           are contiguous in DRAM; row order is free since every row of
           the output shard is identical)
  GpSimd : unused (block exits with no_gpsimd_drain; Pool cannot read
           PSUM -- the BIR verifier rejects it)
HW lessons encoded here (CoreSim models all of these as fine):
  - scalar_tensor_tensor / Activation-engine copies reading PSUM through
    broadcast APs return garbage or abort on HW; only DVE tensor_copy
    handles PSUM + broadcast correctly.
  - A DVE read of a PSUM bank while the PE writes other columns of the
    SAME bank corrupts data -> transposes land in two separate banks.
  - Each dma_start costs ~0.7us engine issue + ~0.8us to first byte, so
    fewer/larger DMAs win; 2-3 KiB descriptors sustain ~200 GB/s.
The store has no explicit completion wait: the block-exit DRAIN on the
issuing engine waits for its HWDGE queue, so the walrus semaphore-reset
epilogue overlaps the output transfer.
"""

import numpy as np
import ml_dtypes

import concourse.bacc as bacc
import concourse.mybir as mybir
from concourse.bass_utils import run_bass_kernel_spmd

B, DIM, CTX_DIM = 4, 512, 768
H = W = 48
NPOS = H * W
NCORES = 8
CPC = DIM // NCORES
P = 128
KC = CTX_DIM // P
KD = DIM // P
ROW = B * CPC
NDUP = 3  # replicas per partition -> 1.5 KiB bf16 DMA descriptors
SELW = B * P + CPC  # sel rows + o5/bias columns
F32 = mybir.dt.float32
BF16 = mybir.dt.bfloat16
BF16NP = ml_dtypes.bfloat16

_CACHE: dict = {}
NWARM = 5  # HAM warmup matmuls; set to 0 before _get_nc() for CoreSim runs


def _build_nc():
    nc = bacc.Bacc("TRN2", target_bir_lowering=False, debug=False, num_devices=NCORES)

    wac = nc.dram_tensor("wac", [P, 2 * DIM + KC * B], BF16, kind="ExternalInput")
    wvc = nc.dram_tensor("wvc", [P, KC, DIM], BF16, kind="ExternalInput")
    woc = nc.dram_tensor("woc", [P, KD, CPC], BF16, kind="ExternalInput")
    idc = nc.dram_tensor("idc", [B, B], F32, kind="ExternalInput")
    smlc = nc.dram_tensor("smlc", [B + 1, SELW], BF16, kind="ExternalInput")
    outd = nc.dram_tensor("outd", [NPOS, ROW], BF16, kind="ExternalOutput")

    wa_sb = nc.alloc_sbuf_tensor("wa_sb", [P, 2 * DIM + KC * B], BF16).ap()
    wv01_v = wa_sb[:, : 2 * DIM].rearrange("p (k d) -> p k d", k=2)
    ctx_v = wa_sb[:, 2 * DIM :].rearrange("p (k b) -> p k b", k=KC)
    wv_sb = nc.alloc_sbuf_tensor("wv_sb", [P, KC, DIM], BF16).ap()
    wo_sb = nc.alloc_sbuf_tensor("wo_sb", [P, KD, CPC], BF16).ap()
    id_sb = nc.alloc_sbuf_tensor("id_sb", [B, B], F32).ap()
    # sml: [:, :B*P] selector rows (viewed [B+1, B, P]); [:, B*P:] o5
    # (rows 0..3 y from PSUM, row 4 the preloaded bias)
    sml_sb = nc.alloc_sbuf_tensor("sml_sb", [B + 1, SELW], BF16).ap()
    sel_v = sml_sb[:, : B * P].rearrange("q (b p) -> q b p", b=B)
    o5_v = sml_sb[:, B * P :]
    t_sb = nc.alloc_sbuf_tensor("t_sb", [B, DIM], F32).ap()
    tT_sb = nc.alloc_sbuf_tensor("tT_sb", [P, KD, B], BF16).ap()
    rep_sb = nc.alloc_sbuf_tensor("rep_sb", [P, NDUP, ROW], BF16).ap()

    pt = nc.alloc_psum_tensor("pt", [B, DIM], F32).ap()
    ptT0 = nc.alloc_psum_tensor("ptT0", [P, KD // 2, B], F32).ap()
    ptT1 = nc.alloc_psum_tensor("ptT1", [P, KD // 2, B], F32).ap()
    po = nc.alloc_psum_tensor("po", [B, CPC], F32).ap()
    prep0 = nc.alloc_psum_tensor("prep0", [P, 2, CPC], F32).ap()
    prep1 = nc.alloc_psum_tensor("prep1", [P, 2, CPC], F32).ap()
    pwarm = nc.alloc_psum_tensor("pwarm", [B, DIM], F32).ap()

    from contextlib import ExitStack

    with ExitStack() as stack:
        # 7 semaphores total: the walrus save/restore pro/epilogue scales
        # with semaphore count, and same-engine increments are ordered, so
        # one monotone counter per producer engine suffices.
        s_wa = stack.enter_context(nc.semaphore("s_wa"))
        s_wv23 = stack.enter_context(nc.semaphore("s_wv23"))
        s_wv45 = stack.enter_context(nc.semaphore("s_wv45"))
        s_const = stack.enter_context(nc.semaphore("s_const"))
        s_p = stack.enter_context(nc.semaphore("s_p"))
        s_v = stack.enter_context(nc.semaphore("s_v"))
        s_out = stack.enter_context(nc.semaphore("s_out"))

        # store view: pos = r*(P*NDUP) + p*NDUP + d, so each partition
        # contributes NDUP consecutive 512-byte rows = 3 KiB contiguous.
        out_view = outd.rearrange("(p r d) n -> p r (d n)", p=P, d=NDUP)
        src_view = (
            rep_sb.rearrange("p d n -> p (d n)")[:, None, :]
            .broadcast_to((P, NPOS // (NDUP * P), NDUP * ROW))
        )
        HP = P // 2

        with nc.Block(no_gpsimd_drain=True) as block:

            @block.sync
            def _(sync):
                sync.dma_start(out=wa_sb[:], in_=wac[:]).then_inc(s_wa, 16)
                sync.dma_start(
                    out=wv_sb[:, 4:6, :], in_=wvc[:, 4:6, :]
                ).then_inc(s_wv45, 16)
                sync.wait_ge(s_v, 7)
                sync.dma_start(out=out_view[:], in_=src_view[:]).then_inc(
                    s_out, 16
                )

            @block.scalar
            def _(scalar):
                scalar.dma_start(
                    out=wv_sb[:, 2:4, :], in_=wvc[:, 2:4, :]
                ).then_inc(s_wv23, 16)
                scalar.dma_start(out=id_sb[:], in_=idc[:]).then_inc(s_const, 16)
                scalar.dma_start(out=wo_sb[:], in_=woc[:]).then_inc(s_const, 16)
                scalar.dma_start(out=sml_sb[:], in_=smlc[:]).then_inc(
                    s_const, 16
                )


            @block.tensor
            def _(tensor):
                # HAM warmup: ungated dummy matmuls (garbage SBUF data,
                # scratch PSUM) keep the PE busy from boot until the first
                # Wv chunk lands, ramping the PE clock from 1.2 to 2.4 GHz.
                for _w in range(NWARM):
                    nc.tensor.matmul(
                        pwarm[:],
                        ctx_v[:, 0, :],
                        wv_sb[:, KC - 1, :],
                        start=(_w == 0),
                        stop=(_w == NWARM - 1),
                    )
                tensor.wait_ge(s_wa, 16)
                gates = [None, None, (s_wv23, 16), None, (s_wv45, 16), None]
                for k in range(KC):
                    if gates[k] is not None:
                        tensor.wait_ge(*gates[k])
                    rhs = wv01_v[:, k, :] if k < 2 else wv_sb[:, k, :]
                    ins = nc.tensor.matmul(
                        pt[:],
                        ctx_v[:, k, :],
                        rhs,
                        start=(k == 0),
                        stop=(k == KC - 1),
                    )
                ins.then_inc(s_p, 1)

                tensor.wait_ge(s_const, 48)
                tensor.wait_ge(s_v, 1)
                for m in range(KD // 2):
                    nc.tensor.transpose(
                        ptT0[:, m, :], t_sb[:, m * P : (m + 1) * P], id_sb[:]
                    ).then_inc(s_p, 1)
                tensor.wait_ge(s_v, 2)
                for m in range(KD // 2, KD):
                    nc.tensor.transpose(
                        ptT1[:, m - KD // 2, :], t_sb[:, m * P : (m + 1) * P],
                        id_sb[:],
                    ).then_inc(s_p, 1)

                # stage2: y = tT.T @ Wo, accumulated over the 4 d-chunks
                tensor.wait_ge(s_v, 3)
                for m in range(KD // 2):
                    nc.tensor.matmul(
                        po[:],
                        tT_sb[:, m, :],
                        wo_sb[:, m, :],
                        start=(m == 0),
                        stop=False,
                    )
                tensor.wait_ge(s_v, 4)
                for m in range(KD // 2, KD):
                    ins = nc.tensor.matmul(
                        po[:],
                        tT_sb[:, m, :],
                        wo_sb[:, m, :],
                        start=False,
                        stop=(m == KD - 1),
                    )
                ins.then_inc(s_p, 1)

                # selector: prep[p, b, :] = y[b, :] + bias, all partitions
                tensor.wait_ge(s_v, 5)
                for b in range(B):
                    pdst = prep0 if b < 2 else prep1
                    ins = nc.tensor.matmul(
                        pdst[:, b % 2, :],
                        sel_v[:, b, :],
                        o5_v[:, :],
                        start=True,
                        stop=True,
                    )
                    if b == 1:
                        ins.then_inc(s_p, 1)
                ins.then_inc(s_p, 1)

            @block.vector
            def _(vector):
                vector.wait_ge(s_p, 1)
                nc.vector.tensor_copy(
                    t_sb[:, : DIM // 2], pt[:, : DIM // 2]
                ).then_inc(s_v, 1)
                nc.vector.tensor_copy(
                    t_sb[:, DIM // 2 :], pt[:, DIM // 2 :]
                ).then_inc(s_v, 1)
                vector.wait_ge(s_p, 3)
                nc.vector.tensor_copy(
                    tT_sb[:, : KD // 2, :], ptT0[:]
                ).then_inc(s_v, 1)
                vector.wait_ge(s_p, 5)
                nc.vector.tensor_copy(
                    tT_sb[:, KD // 2 :, :], ptT1[:]
                ).then_inc(s_v, 1)
                # y rows of o5 (bias row 4 preloaded by the smlc DMA; wait
                # on it so this write is ordered after the DMA's)
                vector.wait_ge(s_p, 6)
                vector.wait_ge(s_const, 48)
                nc.vector.tensor_copy(o5_v[:B, :], po[:]).then_inc(s_v, 1)
                # store source: 6 bias-included replicas per partition, one
                # copy reading prep PSUM through a stride-0 dup dimension
                vector.wait_ge(s_p, 7)
                nc.vector.tensor_copy(
                    rep_sb[:, :, : ROW // 2],
                    prep0.rearrange("p b c -> p (b c)")[:, None, :]
                    .broadcast_to((P, NDUP, ROW // 2)),
                ).then_inc(s_v, 1)
                vector.wait_ge(s_p, 8)
                nc.vector.tensor_copy(
                    rep_sb[:, :, ROW // 2 :],
                    prep1.rearrange("p b c -> p (b c)")[:, None, :]
                    .broadcast_to((P, NDUP, ROW // 2)),
                ).then_inc(s_v, 1)

    nc.compile()
    return nc


def _get_nc():
    if "nc" not in _CACHE:
        _CACHE["nc"] = _build_nc()
    return _CACHE["nc"]


def _prepare_in_maps(context, Wv, Wo, bo):
    context = np.ascontiguousarray(context, dtype=np.float32)
    Wv = np.ascontiguousarray(Wv, dtype=np.float32)
    Wo = np.ascontiguousarray(Wo, dtype=np.float32)
    bo = np.ascontiguousarray(bo, dtype=np.float32)

    ctx_pack = context.T.reshape(KC, P, B).transpose(1, 0, 2).reshape(P, KC * B)
    wv_all = Wv.reshape(KC, P, DIM).transpose(1, 0, 2)
    wac = np.concatenate(
        [wv_all[:, :2, :].reshape(P, 2 * DIM), ctx_pack], axis=1
    ).astype(BF16NP)
    wvc = np.ascontiguousarray(wv_all).astype(BF16NP)
    wo_chunk = Wo.reshape(KD, P, DIM).transpose(1, 0, 2)
    idc = np.eye(B, dtype=np.float32)

    sel = np.zeros((B + 1, B, P), dtype=np.float32)
    for b in range(B):
        sel[b, b, :] = 1.0
        sel[B, b, :] = 1.0

    in_maps = []
    for i in range(NCORES):
        woc = np.ascontiguousarray(
            wo_chunk[:, :, i * CPC : (i + 1) * CPC]
        ).astype(BF16NP)
        smlc = np.zeros((B + 1, SELW), dtype=np.float32)
        smlc[:, : B * P] = sel.reshape(B + 1, B * P)
        smlc[B, B * P :] = bo[i * CPC : (i + 1) * CPC]
        in_maps.append(
            {
                "wac": wac,
                "wvc": wvc,
                "woc": woc,
                "idc": idc,
                "smlc": smlc.astype(BF16NP),
            }
        )
    return in_maps


def _unshard(results):
    shards = np.stack(
        [np.asarray(r["outd"]).astype(np.float32) for r in results], axis=0
    )
    shards = shards.reshape(NCORES, NPOS, B, CPC)
    out = shards.transpose(2, 0, 3, 1).reshape(B, DIM, H, W)
    return np.ascontiguousarray(out)


def kernel(x, context, Wq, Wk, Wv, Wo, bo):
    del x, Wq, Wk
    nc = _get_nc()
    in_maps = _prepare_in_maps(context, Wv, Wo, bo)
    results = run_bass_kernel_spmd(nc, in_maps, list(range(NCORES))).results
    return _unshard(results)
